# revision 15
# baseline (speedup 1.0000x reference)
"""Trainium2 Bass kernel for a 2-layer GAT (nn_GAT_70909910057105).

Strategy (8 NeuronCores, SPMD), v2:
  - Core k owns target nodes [128k, 128k+128). Edges are bucketed by trg//128
    on the host (integer-only preprocessing), then sub-bucketed by src//256 so
    edge-feature rows can be gathered with int16 indices.
  - A bf16 DRAM "node table" holds per-node rows
    [h bf16 x1024 | a_src f32 x16 (bitcast) | pad] (1152 bf16 = 2304B).
    Per-edge source rows are fetched with dma_gather in 1024-row supersteps.
  - All matmuls run in bf16 (f32 is 4 cycles/row on the PE; bf16 is 1).
  - Edge features ship as bf16 and are gathered TRANSPOSED (dma_gather
    transpose=True) so the pe projection needs no PE-array transposes.
  - Per-edge a_tgt comes from maskT @ [at_hi|at_res] matmuls accumulated in
    PSUM and staged to SBUF; these are precomputed under the AllGather
    windows, as is the whole pe pipeline (phase A overlaps AllGather-1,
    pat2 overlaps AllGather-2).
  - segment_sum is a PSUM-accumulated bf16 matmul with host-built one-hot
    masks; the attn scale uses a packed [.., 2]-pair layout to hit the DVE
    2x/4x modes.
  - One AllGather per layer rebuilds the replicated node table.
"""
import sys

for _p in ("/opt/trn_rl_repo", "/root/.axon_site/_ro/trn_rl_repo"):
    if _p not in sys.path:
        sys.path.insert(0, _p)

import numpy as np
import ml_dtypes
import concourse.bass as bass
import concourse.bacc as bacc
import concourse.tile as tile
from concourse import mybir
from concourse.bass_utils import run_bass_kernel_spmd
from concourse.masks import make_identity

F32 = mybir.dt.float32
BF16 = mybir.dt.bfloat16
I16 = mybir.dt.int16
F16 = mybir.dt.float16
NPBF = ml_dtypes.bfloat16

N, B, C, H, D = 1024, 4, 256, 4, 64
E = 32768
NC = 8
TPC = N // NC           # target nodes per core = 128
ROW = 1152              # bf16 elems: 1024 h | 32 (16 f32 a_src) | 96 pad
AS_OFF = 1024           # bf16-elem offset of a_src f32 region
NB_LOCAL = TPC * B      # 512 local (node, batch) rows
Q = 4                   # src quarters (int16 edge-feature indexing)
QROWS = (N // Q) * TPC  # 32768 rows per edge-feature shard quarter
SUP = 1024              # edges per table-gather superstep


# --------------------------------------------------------------------------
# host-side preprocessing (integer / layout ops only)
# --------------------------------------------------------------------------

def _pack_idx(vals: np.ndarray) -> np.ndarray:
    n = vals.shape[0]
    assert n % 16 == 0
    blk = vals.astype(np.int16).reshape(n // 16, 16).T
    return np.ascontiguousarray(np.tile(blk, (8, 1)))


def _prep(x, edge_features, src_idx, trg_idx,
          Wn1, We1, a_src1, a_tgt1, a_edge1,
          Wn2, We2, a_src2, a_tgt2, a_edge2):
    src = np.asarray(src_idx).astype(np.int64)
    trg = np.asarray(trg_idx).astype(np.int64)
    x = np.asarray(x, dtype=np.float32)
    ef = np.asarray(edge_features)

    per_core = []
    bmax = 0
    for k in range(NC):
        eids = np.nonzero((trg // TPC) == k)[0]
        bks = [eids[(src[eids] // (N // Q)) == q] for q in range(Q)]
        per_core.append(bks)
        bmax = max(bmax, max(len(b) for b in bks))
    B_pad = ((bmax + 127) // 128) * 128
    E_pad = Q * B_pad

    xf = x.reshape(N * B, C)
    xT = np.ascontiguousarray(xf.T)

    def sb3f(w, inner):
        return np.ascontiguousarray(
            w.reshape(2, 128, inner).transpose(1, 0, 2))

    def sb3(w, inner):
        return sb3f(w, inner).astype(NPBF)

    def hself(a_e):
        m = np.zeros((C, H), np.float32)
        for h in range(H):
            m[h * D:(h + 1) * D, h] = np.float32(a_e[h])
        return sb3f(m, H)

    def ablkf(a_s, a_t):
        m = np.zeros((C, 2 * H), np.float32)
        for h in range(H):
            m[h * D:(h + 1) * D, h] = np.asarray(a_s)[h]
            m[h * D:(h + 1) * D, H + h] = np.asarray(a_t)[h]
        return sb3f(m, 2 * H)

    common = {
        "wn1hd": sb3f(np.asarray(Wn1, np.float32), C),
        "wn2hd": sb3f(np.asarray(Wn2, np.float32), C),
        "wn1cols": sb3f(np.ascontiguousarray(np.asarray(Wn1, np.float32).T), C),
        "wn2cols": sb3(np.ascontiguousarray(np.asarray(Wn2, np.float32).T), C),
        "we1hd": sb3f(np.asarray(We1, np.float32), C),
        "we2hd": sb3f(np.asarray(We2, np.float32), C),
        "hsel1": hself(np.asarray(a_edge1)),
        "hsel2": hself(np.asarray(a_edge2)),
        "ablk1": ablkf(a_src1, a_tgt1),
        "ablk2": ablkf(a_src2, a_tgt2),
    }

    in_maps = []
    for k in range(NC):
        src_s = np.zeros(E_pad, np.int64)
        efi_s = np.zeros(E_pad, np.int64)
        mask = np.zeros((128, E_pad), np.float32)
        maskT = np.zeros((128, E_pad), np.float32)
        for q in range(Q):
            ids = per_core[k][q]
            s0 = q * B_pad
            src_s[s0:s0 + len(ids)] = src[ids]
            tl = trg[ids] - k * TPC
            efi_s[s0:s0 + len(ids)] = (src[ids] - q * (N // Q)) * TPC + tl
            for sslot, t in zip(range(s0, s0 + len(ids)), tl):
                mask[sslot % 128, (sslot // 128) * 128 + t] = 1.0
                maskT[t, (sslot // 128) * 128 + sslot % 128] = 1.0
        shard = np.ascontiguousarray(
            ef[:, k * TPC:(k + 1) * TPC, :]).reshape(N * TPC, C).astype(np.float16)
        m = dict(common)
        m.update({
            "ef": shard,
            "xT": np.ascontiguousarray(
                xT[:, k * NB_LOCAL:(k + 1) * NB_LOCAL]
            ).reshape(2, 128, NB_LOCAL).transpose(1, 0, 2).copy(),
            "isrc": _pack_idx(src_s),
            "ief": _pack_idx(efi_s),
            "mask": mask.astype(NPBF),
            "maskT": maskT.astype(NPBF),
        })
        in_maps.append(m)
    return in_maps, B_pad, E_pad, E_pad // 128


# --------------------------------------------------------------------------
# device program
# --------------------------------------------------------------------------

def _build(B_pad: int):
    E_pad = Q * B_pad
    n_chunks = E_pad // 128
    # supersteps of up to SUP edges (8 chunks); last may be 4 chunks
    supers = []
    c0 = 0
    while c0 < n_chunks:
        nci = min(SUP // 128, n_chunks - c0)
        supers.append((c0, nci))
        c0 += nci
    nc = bacc.Bacc("TRN2", target_bir_lowering=False, debug=False,
                   num_devices=NC)

    ef_in = nc.dram_tensor("ef", [Q * QROWS, C], F16, kind="ExternalInput")
    xT_in = nc.dram_tensor("xT", [128, 2, NB_LOCAL], F32, kind="ExternalInput")
    isrc_in = nc.dram_tensor("isrc", [128, E_pad // 16], I16, kind="ExternalInput")
    ief_in = nc.dram_tensor("ief", [128, E_pad // 16], I16, kind="ExternalInput")
    mask_in = nc.dram_tensor("mask", [128, E_pad], BF16, kind="ExternalInput")
    maskT_in = nc.dram_tensor("maskT", [128, E_pad], BF16, kind="ExternalInput")
    w_dt = {"wn2cols": BF16}
    w_in = {
        nm: nc.dram_tensor(nm, [128, 2, inner], w_dt.get(nm, F32),
                           kind="ExternalInput")
        for nm, inner in [
            ("wn1hd", C), ("wn2hd", C), ("wn1cols", C), ("wn2cols", C),
            ("we1hd", C), ("we2hd", C),
            ("hsel1", H), ("hsel2", H), ("ablk1", 2 * H), ("ablk2", 2 * H),
        ]
    }
    y_out = nc.dram_tensor("y", [128, B * C], F32, kind="ExternalOutput")

    from contextlib import ExitStack
    with tile.TileContext(nc) as tc:
        with ExitStack() as ctx:
            const = ctx.enter_context(tc.tile_pool(name="const", bufs=1))
            sb = ctx.enter_context(tc.tile_pool(name="sb", bufs=1))
            small = ctx.enter_context(tc.tile_pool(name="small", bufs=3))
            gpool = ctx.enter_context(tc.tile_pool(name="gpool", bufs=3))
            ps_small = ctx.enter_context(
                tc.tile_pool(name="ps_small", bufs=2, space="PSUM"))
            ps_t = ctx.enter_context(
                tc.tile_pool(name="ps_t", bufs=2, space="PSUM"))
            ps_out = ctx.enter_context(
                tc.tile_pool(name="ps_out", bufs=1, space="PSUM"))
            ps_den = ctx.enter_context(
                tc.tile_pool(name="ps_den", bufs=1, space="PSUM"))
            dram = ctx.enter_context(tc.tile_pool(name="dram", bufs=1, space="DRAM"))

            ident = const.tile([128, 128], BF16)
            make_identity(nc, ident[:])
            zpad = const.tile([128, 24], BF16)
            nc.vector.memset(zpad[:], 0.0)

            w_sb = {}
            for nm, t in w_in.items():
                inner = t.shape[2]
                w_sb[nm] = const.tile([128, 2, inner], w_dt.get(nm, F32),
                                      name=f"w_{nm}", tag=f"w_{nm}")
                nc.sync.dma_start(out=w_sb[nm][:], in_=t[:])
            xT_sb = const.tile([128, 2, NB_LOCAL], F32)
            nc.sync.dma_start(out=xT_sb[:], in_=xT_in[:])
            isrc_t = const.tile([128, E_pad // 16], I16)
            nc.sync.dma_start(out=isrc_t[:], in_=isrc_in[:])
            ief_t = const.tile([128, E_pad // 16], I16)
            nc.sync.dma_start(out=ief_t[:], in_=ief_in[:])
            mask_sb = const.tile([128, E_pad], BF16)
            nc.sync.dma_start(out=mask_sb[:], in_=mask_in[:])
            maskT_sb = const.tile([128, E_pad], BF16)
            nc.sync.dma_start(out=maskT_sb[:], in_=maskT_in[:])

            # ---- wesum / A prep (f32 matmuls, rounded once for bf16 users)
            wesum_sb = const.tile([128, 2, 2 * H], F16)
            a1_sb = const.tile([128, 2, 2 * H], F32)   # layer-1 build is f32
            a2_f = const.tile([128, 2, 2 * H], F32)
            a2_hi = const.tile([128, 2, 2 * H], BF16)  # layer-2 build is bf16
            a2_res = const.tile([128, 2, 2 * H], BF16)
            a2_tmp = const.tile([128, 2, 2 * H], F32)
            for ct in range(2):
                pw = ps_small.tile([128, 2 * H], F32, space="PSUM", tag="ps", name="pw")
                for lj, (wehd, hs) in enumerate(
                        [("we1hd", "hsel1"), ("we2hd", "hsel2")]):
                    for kh in range(2):
                        nc.tensor.matmul(
                            out=pw[:, lj * H:(lj + 1) * H],
                            lhsT=w_sb[wehd][:, kh, ct * 128:(ct + 1) * 128],
                            rhs=w_sb[hs][:, kh, :],
                            start=(kh == 0), stop=(kh == 1))
                nc.scalar.copy(out=wesum_sb[:, ct, :], in_=pw[:])
                for dst, wnhd, ab in [(a1_sb, "wn1hd", "ablk1"),
                                      (a2_f, "wn2hd", "ablk2")]:
                    pa = ps_small.tile([128, 2 * H], F32, space="PSUM", tag="ps", name="pa")
                    for kh in range(2):
                        nc.tensor.matmul(
                            out=pa[:],
                            lhsT=w_sb[wnhd][:, kh, ct * 128:(ct + 1) * 128],
                            rhs=w_sb[ab][:, kh, :],
                            start=(kh == 0), stop=(kh == 1))
                    nc.scalar.copy(out=dst[:, ct, :], in_=pa[:])
            nc.vector.tensor_copy(out=a2_hi[:], in_=a2_f[:])
            nc.vector.tensor_copy(out=a2_tmp[:], in_=a2_hi[:])
            nc.vector.tensor_tensor(out=a2_tmp[:], in0=a2_f[:], in1=a2_tmp[:],
                                    op=mybir.AluOpType.subtract)
            nc.vector.tensor_copy(out=a2_res[:], in_=a2_tmp[:])

            # ---- local table build (h + a_src rows; a_tgt hi/res rhs local)
            # a_rhs: list of accumulating rhs operands (1 f32 or 2 bf16 hi/res)
            def build_table(lhsT_sb, wncols, a_rhs, tag):
                ag_in = dram.tile([TPC, ROW], BF16, tag=f"agin{tag}",
                                  name=f"agin{tag}")
                table = dram.tile([N, ROW], BF16, addr_space="Shared",
                                  tag=f"tbl{tag}", name=f"tbl{tag}")
                na = len(a_rhs)
                for t in range(4):
                    ph = ps_small.tile([128, C], F32, space="PSUM", tag="ps", name="ph")
                    pa = ps_small.tile([128, 2 * H], F32, space="PSUM", tag="ps", name="pa2")
                    for ch in range(2):
                        lhsT = lhsT_sb[:, ch, t * 128:(t + 1) * 128]
                        nc.tensor.matmul(out=ph[:], lhsT=lhsT,
                                         rhs=wncols[:, ch, :],
                                         start=(ch == 0), stop=(ch == 1))
                        for ia, ar in enumerate(a_rhs):
                            nc.tensor.matmul(
                                out=pa[:], lhsT=lhsT, rhs=ar[:, ch, :],
                                start=(ch == 0 and ia == 0),
                                stop=(ch == 1 and ia == na - 1))
                    sh = small.tile([128, C], BF16, tag="sh")
                    sa = small.tile([128, 2 * H], F32, tag="sa")
                    nc.scalar.copy(out=sh[:], in_=ph[:])
                    nc.scalar.copy(out=sa[:], in_=pa[:])
                    rows = slice(t * 32, (t + 1) * 32)
                    nc.sync.dma_start(
                        out=ag_in[rows, 0:B * C].rearrange(
                            "n (b o) -> n b o", b=B),
                        in_=sh[:])
                    nc.sync.dma_start(
                        out=ag_in[rows, AS_OFF:AS_OFF + 2 * B * H].bitcast(
                            F32).rearrange("n (b h) -> n b h", b=B),
                        in_=sa[:, 0:H])
                    nc.sync.dma_start(
                        out=ag_in[rows, AS_OFF + 2 * B * H:ROW].rearrange(
                            "n (b z) -> n b z", b=B),
                        in_=zpad[:])
                # local a_tgt[t, (b h)] via per-b matmuls, then hi/res split
                at_loc = small.tile([128, B * H], F32, tag="atl")
                for b in range(B):
                    pab = ps_small.tile([128, 2 * H], F32, space="PSUM",
                                        tag="ps", name="pab")
                    for ch in range(2):
                        lhsT_b = lhsT_sb[:, ch, :].rearrange(
                            "p (n b2) -> p b2 n", b2=B)[:, b, :]
                        for ia, ar in enumerate(a_rhs):
                            nc.tensor.matmul(
                                out=pab[:], lhsT=lhsT_b, rhs=ar[:, ch, :],
                                start=(ch == 0 and ia == 0),
                                stop=(ch == 1 and ia == na - 1))
                    nc.vector.tensor_copy(out=at_loc[:, b * H:(b + 1) * H],
                                          in_=pab[:, H:2 * H])
                at_rhs = small.tile([128, 2 * B * H], BF16, tag="atr")
                at_tmp = small.tile([128, B * H], F32, tag="att")
                nc.vector.tensor_copy(out=at_rhs[:, 0:B * H], in_=at_loc[:])
                nc.vector.tensor_copy(out=at_tmp[:], in_=at_rhs[:, 0:B * H])
                nc.vector.tensor_tensor(out=at_tmp[:], in0=at_loc[:],
                                        in1=at_tmp[:],
                                        op=mybir.AluOpType.subtract)
                nc.vector.tensor_copy(out=at_rhs[:, B * H:2 * B * H],
                                      in_=at_tmp[:])
                nc.gpsimd.collective_compute(
                    "AllGather", mybir.AluOpType.bypass,
                    replica_groups=[list(range(NC))],
                    ins=[ag_in.opt()], outs=[table.opt()])
                return table, at_rhs

            # ---- pat precompute: scores_sb[e_chunk, (b h)] = a_tgt per edge
            def pat_all(at_rhs, tag):
                scores = sb.tile([128, n_chunks, B * H], F32,
                                 tag=f"sc{tag}", name=f"sc{tag}")
                for g in range(n_chunks // 4):
                    pp = ps_small.tile([128, 4, B * H], F32, space="PSUM",
                                       tag="ps", name="pp")
                    for jc in range(4):
                        c = g * 4 + jc
                        mt = maskT_sb[:, c * 128:(c + 1) * 128]
                        nc.tensor.matmul(out=pp[:, jc, :], lhsT=mt,
                                         rhs=at_rhs[:, 0:B * H],
                                         start=True, stop=False)
                        nc.tensor.matmul(out=pp[:, jc, :], lhsT=mt,
                                         rhs=at_rhs[:, B * H:2 * B * H],
                                         start=False, stop=True)
                    nc.scalar.copy(out=scores[:, g * 4:(g + 1) * 4, :],
                                   in_=pp[:])
                return scores

            # ---- phase A: transposed ef gather + pe[e, (layer h)] matmuls
            def phase_a():
                eTs = []
                for q in range(Q):
                    eT = sb.tile([128, 2, B_pad], F16, tag=f"eT{q}",
                                 name=f"eT{q}")
                    nc.gpsimd.dma_gather(
                        out_ap=eT[:],
                        in_ap=ef_in[q * QROWS:(q + 1) * QROWS, :],
                        idxs_ap=ief_t[:, q * (B_pad // 16):(q + 1) * (B_pad // 16)],
                        num_idxs=B_pad, num_idxs_reg=B_pad, elem_size=C,
                        transpose=True, single_packet=False)
                    eTs.append(eT)
                cpq = B_pad // 128  # chunks per quarter
                pe_sb = sb.tile([128, n_chunks, 2 * H], F32)
                for g in range(n_chunks // 4):
                    pp = ps_small.tile([128, 4, 2 * H], F32, space="PSUM",
                                       tag="ps", name="ppe")
                    for jc in range(4):
                        c = g * 4 + jc
                        eT = eTs[c // cpq]
                        cl = c % cpq
                        for ch in range(2):
                            nc.tensor.matmul(
                                out=pp[:, jc, :],
                                lhsT=eT[:, ch, cl * 128:(cl + 1) * 128],
                                rhs=wesum_sb[:, ch, :],
                                start=(ch == 0), stop=(ch == 1))
                    nc.scalar.copy(out=pe_sb[:, g * 4:(g + 1) * 4, :],
                                   in_=pp[:])
                return pe_sb

            # ---- edge loop for one layer
            def edge_loop(table, scores, pe_sb, layer, out_bf16):
                out_p = ps_out.tile([128, B * C], F32, space="PSUM", tag="out",
                                    name="out_p")
                den_p = ps_den.tile([128, B * H], F32, space="PSUM", tag="den",
                                    name="den_p")
                for (sc0, nci) in supers:
                    n_e = nci * 128
                    G = gpool.tile([128, SUP // 128, ROW], BF16, tag="G")
                    nc.gpsimd.dma_gather(
                        out_ap=G[:, 0:nci, :], in_ap=table[:],
                        idxs_ap=isrc_t[:, sc0 * 8:sc0 * 8 + n_e // 16],
                        num_idxs=n_e, num_idxs_reg=n_e, elem_size=ROW,
                        single_packet=False)
                    s_sb = small.tile([128, SUP // 128, B * H], F32, tag="s")
                    t_sb = small.tile([128, SUP // 128, B * H], F32, tag="t")
                    e2 = small.tile([128, SUP // 128, 2 * B * H], BF16, tag="e2")
                    nc.vector.tensor_tensor(
                        out=s_sb[:, 0:nci, :],
                        in0=scores[:, sc0:sc0 + nci, :],
                        in1=G[:, 0:nci, AS_OFF:AS_OFF + 2 * B * H].bitcast(F32),
                        op=mybir.AluOpType.add)
                    nc.vector.tensor_tensor(
                        out=s_sb[:, 0:nci, :].rearrange(
                            "p c (b h) -> p c b h", b=B),
                        in0=s_sb[:, 0:nci, :].rearrange(
                            "p c (b h) -> p c b h", b=B),
                        in1=pe_sb[:, sc0:sc0 + nci, layer * H:(layer + 1) * H]
                            .rearrange("p c (u h) -> p c u h", u=1)
                            .to_broadcast([128, nci, B, H]),
                        op=mybir.AluOpType.add)
                    nc.scalar.mul(out=t_sb[:, 0:nci, :],
                                  in_=s_sb[:, 0:nci, :], mul=0.2)
                    nc.vector.tensor_tensor(
                        out=s_sb[:, 0:nci, :], in0=s_sb[:, 0:nci, :],
                        in1=t_sb[:, 0:nci, :], op=mybir.AluOpType.max)
                    e2v = e2[:, 0:nci, :].rearrange(
                        "p c (e two) -> p c two e", two=2)
                    for half in range(2):
                        nc.scalar.activation(
                            out=e2v[:, :, half, :], in_=s_sb[:, 0:nci, :],
                            func=mybir.ActivationFunctionType.Exp)
                    for j in range(nci):
                        c = sc0 + j
                        nc.vector.tensor_tensor(
                            out=G[:, j, 0:B * C].rearrange(
                                "p (e d two) -> p e d two", e=B * H, two=2),
                            in0=G[:, j, 0:B * C].rearrange(
                                "p (e d two) -> p e d two", e=B * H, two=2),
                            in1=e2[:, j, :].rearrange(
                                "p (e u two) -> p e u two", u=1, two=2)
                                .to_broadcast([128, B * H, D // 2, 2]),
                            op=mybir.AluOpType.mult)
                        mk = mask_sb[:, c * 128:(c + 1) * 128]
                        first, last = (c == 0), (c == n_chunks - 1)
                        nc.tensor.matmul(out=out_p[:, 0:512], lhsT=mk,
                                         rhs=G[:, j, 0:512],
                                         start=first, stop=last)
                        nc.tensor.matmul(out=out_p[:, 512:1024], lhsT=mk,
                                         rhs=G[:, j, 512:1024],
                                         start=first, stop=last)
                        nc.tensor.matmul(out=den_p[:], lhsT=mk,
                                         rhs=e2[:, j, :].rearrange(
                                             "p (e two) -> p two e", two=2)[:, 0, :],
                                         start=first, stop=last)
                dsb = small.tile([128, B * H], F32, tag="d")
                nc.vector.tensor_scalar_add(dsb[:], den_p[:], 1e-16)
                rec = small.tile([128, B * H], F32, tag="r")
                nc.vector.reciprocal(rec[:], dsb[:])
                xo = sb.tile([128, B * C], BF16 if out_bf16 else F32,
                             tag=f"xo{layer}", name=f"xo{layer}")
                nc.vector.tensor_tensor(
                    out=xo[:].rearrange("p (x d) -> p x d", d=D),
                    in0=out_p[:].rearrange("p (x d) -> p x d", d=D),
                    in1=rec[:].rearrange("p (x u) -> p x u", u=1)
                        .to_broadcast([128, B * H, D]),
                    op=mybir.AluOpType.mult)
                return xo

            # ---- program order chosen so phase A / pat overlap AllGathers
            table1, at1 = build_table(xT_sb, w_sb["wn1cols"], [a1_sb], 1)
            pe_sb = phase_a()
            sc1 = pat_all(at1, 1)
            x1 = edge_loop(table1, sc1, pe_sb, 0, out_bf16=True)

            x1T = sb.tile([128, 2, NB_LOCAL], BF16)
            for b in range(B):
                for ch in range(2):
                    pt = ps_t.tile([128, 128], BF16, space="PSUM", tag="pt",
                                   name="pt")
                    nc.tensor.transpose(
                        out=pt[:],
                        in_=x1[:, b * C + ch * 128: b * C + (ch + 1) * 128],
                        identity=ident[:])
                    nc.scalar.copy(
                        out=x1T[:, ch, :].rearrange(
                            "p (n b2) -> p n b2", b2=B)[:, :, b],
                        in_=pt[:])

            table2, at2 = build_table(x1T, w_sb["wn2cols"], [a2_hi, a2_res], 2)
            sc2 = pat_all(at2, 2)
            x2 = edge_loop(table2, sc2, pe_sb, 1, out_bf16=False)
            nc.sync.dma_start(out=y_out[:], in_=x2[:])

    nc.compile()
    return nc


_CACHE: dict = {}


def _get_program(B_pad: int):
    if B_pad not in _CACHE:
        _CACHE[B_pad] = _build(B_pad)
    return _CACHE[B_pad]


def kernel(debug=False, trace=False, **inputs):
    in_maps, B_pad, E_pad, n_chunks = _prep(**inputs)
    nc = _get_program(B_pad)
    res = run_bass_kernel_spmd(nc, in_maps, core_ids=list(range(NC)),
                               trace=trace)
    y = np.concatenate([res.results[k]["y"] for k in range(NC)], axis=0)
    out = y.reshape(N, B, C)
    if debug or trace:
        return out, res
    return out
